# revision 22
# baseline (speedup 1.0000x reference)
"""Trainium2 Bass kernel for nn_MessageFunction (gnn_message_passing).

Math (validated against the reference):
  The reference broadcasts h_w[:, :, None] -> (B*N, IN_F, N) and reshapes to
  [E, IN_F]; row-major order makes every row constant:
      h_w_rows[e, i] = h_w.reshape(-1)[e]   for all i.
  Hence the per-edge bmm collapses:
      m[e, o] = sum_i edge_output[e, o, i] * s[e]
              = s[e] * (x3[e] @ W4s[:, o] + b4s[o])
  with W4s = W4.reshape(HID3, OUT_F, IN_F).sum(-1), b4s = b4.reshape(OUT_F,
  IN_F).sum(-1), s = h_w.reshape(-1).  This is an exact reassociation (only
  f32 rounding differences) and removes the [E,128]@[128,4096] matmul + bmm.

Kernel: data-parallel over E = 32768 edges, 4096 per core across 8 cores,
MLP weights replicated, no cross-core communication.  Per core the MLP runs
features-on-partitions with edges streaming on the free dim:
    x1 = relu(W1.T @ eT)        K=32  -> [128, e]
    x2 = relu(W2.T @ x1)        K=128 -> [256, e] (two 128-part halves)
    x3 = relu(W3.T @ x2)        K=256 -> [128, e] (PSUM accumulation)
    y  = W4s.T @ x3             K=128 -> [64, e]  (col-packed 2 tiles/PSUM)
    out = (y + b4s) * s         one fused scalar_tensor_tensor on VectorE
Matmuls use float32r (full PE rate at N=512, near-fp32 precision).
"""

import os

import numpy as np

import concourse.bacc as bacc
import concourse.bass as bass
import concourse.mybir as mybir
import concourse.tile as tile
from concourse.bass_utils import run_bass_kernel_spmd

# Problem constants (hardcoded per the harness contract).
B, N = 8, 64
IN_F, OUT_F = 64, 64
EDGE_F = 32
HID1, HID2, HID3 = 128, 256, 128
E = B * N * N            # 32768
N_CORES = 8
E_LOC = E // N_CORES     # 4096
TILE = 512               # edges per tile (one PSUM bank per stage)
NT = E_LOC // TILE       # 8 tiles per core
OUT_CHUNK = 1024         # output DMA granularity (2 tiles)

F32 = mybir.dt.float32
# Matmul operand dtype: float32r streams at 1 cycle/row for N>=256 (same as
# bf16) with much better precision than bf16.
DT = mybir.dt.float32r
NP_DT = np.float32

# Module global: last BassKernelResults (test.py reads exec_time_ns from it).
LAST_RESULTS = None


def _build_bass(b2_halves_equal=True):
    nc = bacc.Bacc(
        "TRN2", target_bir_lowering=False, debug=False, num_devices=N_CORES
    )

    # Per-core inputs
    e_t = nc.dram_tensor("e_t", [EDGE_F, E_LOC], DT, kind="ExternalInput")
    s_b = nc.dram_tensor("s_b", [OUT_F, E_LOC], F32, kind="ExternalInput")
    # Replicated weights
    w1d = nc.dram_tensor("w1d", [EDGE_F, HID1], DT, kind="ExternalInput")
    w2d = nc.dram_tensor("w2d", [HID1, HID2], DT, kind="ExternalInput")
    # W3 packed side by side: [:, 0:128] = W3[0:128, :], [:, 128:256] = W3[128:256, :]
    w3d = nc.dram_tensor("w3d", [128, 2 * HID3], DT, kind="ExternalInput")
    w4d = nc.dram_tensor("w4d", [HID3, OUT_F], DT, kind="ExternalInput")
    # Bias columns: b1, b2[:128], b2[128:], b3, [b4s; pad]
    bbd = nc.dram_tensor("bbd", [128, 5], F32, kind="ExternalInput")
    outd = nc.dram_tensor(
        "outd", [E_LOC // OUT_CHUNK, OUT_F, OUT_CHUNK], F32, kind="ExternalOutput"
    )

    # Relu pass engine schedule (per tile: L1, L2a, L2b, L3). 'A' = ScalarE,
    # 'V' = VectorE.  VectorE also runs the eight final bias+scale ops, so
    # ScalarE takes more of the 32 relu passes (20 A / 12 V).
    relu_sched = ["AVAV", "AVAA"] * (NT // 2)

    with tile.TileContext(nc) as tc:
        with (
            tc.tile_pool(name="wp", bufs=1) as wp,
            tc.tile_pool(name="io", bufs=4) as io,
            tc.tile_pool(name="acts", bufs=3) as acts,
            tc.tile_pool(name="ps", bufs=1, space="PSUM") as ps,
        ):
            w1 = wp.tile([EDGE_F, HID1], DT, tag="w1")
            w2 = wp.tile([HID1, HID2], DT, tag="w2")
            w3 = wp.tile([128, 2 * HID3], DT, tag="w3")
            w4 = wp.tile([HID3, OUT_F], DT, tag="w4")
            bb = wp.tile([128, 5], F32, tag="bb")
            s_sb = wp.tile([OUT_F, E_LOC], F32, tag="s_sb")
            out_sb = wp.tile([OUT_F, E_LOC], F32, tag="out_sb")
            nc.sync.dma_start(w1[:], w1d[:])
            nc.sync.dma_start(w2[:], w2d[:])
            nc.sync.dma_start(w3[:], w3d[:])
            nc.sync.dma_start(w4[:], w4d[:])
            nc.sync.dma_start(bb[:], bbd[:])
            nc.sync.dma_start(s_sb[:], s_b[:])

            def relu_pass(dst, src, bias_col, eng):
                if eng == "A":
                    nc.scalar.activation(
                        dst, src, mybir.ActivationFunctionType.Relu, bias=bias_col
                    )
                else:
                    nc.vector.tensor_scalar(
                        out=dst,
                        in0=src,
                        scalar1=bias_col,
                        scalar2=0.0,
                        op0=mybir.AluOpType.add,
                        op1=mybir.AluOpType.max,
                    )

            # Software-pipelined emission, skewed so each pass result is
            # consumed one full iteration after it is produced — the PE
            # matmul stream never waits on a just-issued ScalarE/VectorE
            # pass.  Stage s of tile t runs in iteration t+s.
            et_t = [None] * NT
            x1_t = [None] * NT
            x2_t = [None] * NT
            x3_t = [None] * NT
            # P3 engine: 5 on ScalarE / 3 on VectorE (balance against the
            # fixed STT work on VectorE); P1 on VectorE, P2 on ScalarE.
            p3_eng = ["A", "V", "A", "A", "V", "A", "A", "V"]

            for i in range(NT + 4):
                # Stage DMA: prefetch et(i)
                if i < NT:
                    et = io.tile([EDGE_F, TILE], DT, tag="et")
                    nc.sync.dma_start(et[:], e_t[:, i * TILE : (i + 1) * TILE])
                    et_t[i] = et

                # S1 + P1 for tile i (uses the et DMA'd this iteration; the
                # DMA is small and issued at iteration start)
                if 0 <= i < NT:
                    x1p = ps.tile([128, TILE], F32, tag="x1p", bufs=2)
                    nc.tensor.matmul(x1p[:], w1[:], et_t[i][:])
                    x1 = acts.tile([128, TILE], DT, tag="x1")
                    relu_pass(x1[:], x1p[:], bb[:, 0:1], "V")
                    x1_t[i] = x1

                # S2 + P2 for tile i-1 (merged 2-bank PSUM, single pass)
                j = i - 1
                if 0 <= j < NT:
                    x2p = ps.tile([128, 2 * TILE], F32, tag="x2p", bufs=1)
                    nc.tensor.matmul(x2p[:, 0:TILE], w2[:, 0:128], x1_t[j][:])
                    nc.tensor.matmul(x2p[:, TILE : 2 * TILE], w2[:, 128:256], x1_t[j][:])
                    x2 = acts.tile([128, 2 * TILE], DT, tag="x2")
                    # A per-partition bias is constant along the free dim, so
                    # one merged pass is only valid when both b2 halves agree
                    # (always true for the zero biases here); otherwise fall
                    # back to two passes.
                    if b2_halves_equal:
                        nc.scalar.activation(
                            x2[:], x2p[:],
                            mybir.ActivationFunctionType.Relu, bias=bb[:, 1:2],
                        )
                    else:
                        nc.scalar.activation(
                            x2[:, 0:TILE], x2p[:, 0:TILE],
                            mybir.ActivationFunctionType.Relu, bias=bb[:, 1:2],
                        )
                        nc.scalar.activation(
                            x2[:, TILE : 2 * TILE], x2p[:, TILE : 2 * TILE],
                            mybir.ActivationFunctionType.Relu, bias=bb[:, 2:3],
                        )
                    x2_t[j] = x2
                    x1_t[j] = None

                # S3 + P3 for tile i-2
                j = i - 2
                if 0 <= j < NT:
                    x3ps = ps.tile([128, TILE], F32, tag="x3ps", bufs=2)
                    nc.tensor.matmul(
                        x3ps[:], w3[:, 0:128], x2_t[j][:, 0:TILE],
                        start=True, stop=False,
                    )
                    nc.tensor.matmul(
                        x3ps[:], w3[:, 128:256], x2_t[j][:, TILE : 2 * TILE],
                        start=False, stop=True,
                    )
                    x3 = acts.tile([128, TILE], DT, tag="x3")
                    relu_pass(x3[:], x3ps[:], bb[:, 3:4], p3_eng[j])
                    x3_t[j] = x3
                    x2_t[j] = None

                # S4 + P4 for tile i-3
                j = i - 3
                if 0 <= j < NT:
                    cs = slice(j * TILE, (j + 1) * TILE)
                    y4p = ps.tile([OUT_F, TILE], F32, tag="y4p", bufs=2)
                    nc.tensor.matmul(y4p[:], w4[:], x3_t[j][:])
                    nc.vector.scalar_tensor_tensor(
                        out=out_sb[:, cs],
                        in0=y4p[:],
                        scalar=bb[0:OUT_F, 4:5],
                        in1=s_sb[:, cs],
                        op0=mybir.AluOpType.add,
                        op1=mybir.AluOpType.mult,
                    )
                    x3_t[j] = None
                    if (j + 1) * TILE % OUT_CHUNK == 0:
                        ck = ((j + 1) * TILE) // OUT_CHUNK - 1
                        nc.sync.dma_start(
                            outd[ck],
                            out_sb[:, ck * OUT_CHUNK : (ck + 1) * OUT_CHUNK],
                        )

    nc.compile()
    return nc


_CACHED_NC = None


def kernel(h_v, h_w, e_vw, W1, b1, W2, b2, W3, b3, W4, b4):
    global LAST_RESULTS, _CACHED_NC

    h_w = np.asarray(h_w, np.float32)
    e_vw = np.asarray(e_vw, np.float32)
    W1 = np.asarray(W1, np.float32)
    W2 = np.asarray(W2, np.float32)
    W3 = np.asarray(W3, np.float32)
    W4 = np.asarray(W4, np.float32)
    b1 = np.asarray(b1, np.float32)
    b2 = np.asarray(b2, np.float32)
    b3 = np.asarray(b3, np.float32)
    b4 = np.asarray(b4, np.float32)

    # Host-side weight transform (exact reassociation of the reference math).
    W4s = W4.reshape(HID3, OUT_F, IN_F).sum(axis=2)
    b4s = b4.reshape(OUT_F, IN_F).sum(axis=1)
    s = h_w.reshape(-1)

    w3p = np.concatenate([W3[0:128], W3[128:256]], axis=1)  # [128, 256]
    bb = np.zeros((128, 5), np.float32)
    bb[:, 0] = b1
    bb[:, 1] = b2[0:128]
    bb[:, 2] = b2[128:256]
    bb[:, 3] = b3
    bb[0:OUT_F, 4] = b4s

    weights_map = {
        "w1d": np.ascontiguousarray(W1, NP_DT),
        "w2d": np.ascontiguousarray(W2, NP_DT),
        "w3d": np.ascontiguousarray(w3p, NP_DT),
        "w4d": np.ascontiguousarray(W4s, NP_DT),
        "bbd": bb,
    }

    in_maps = []
    for c in range(N_CORES):
        sl = slice(c * E_LOC, (c + 1) * E_LOC)
        e_loc = e_vw[sl]                       # [4096, 32]
        s_loc = s[sl]                          # [4096]
        e_t = np.ascontiguousarray(e_loc.T, NP_DT)   # [32, 4096]
        s_bcast = np.ascontiguousarray(
            np.broadcast_to(s_loc[None, :], (OUT_F, E_LOC)), np.float32
        )
        in_maps.append({"e_t": e_t, "s_b": s_bcast, **weights_map})

    if _CACHED_NC is None:
        _CACHED_NC = _build_bass(
            b2_halves_equal=bool(np.array_equal(b2[0:128], b2[128:256]))
        )
    nc = _CACHED_NC

    trace = bool(int(os.environ.get("KERNEL_TRACE", "0")))
    res = run_bass_kernel_spmd(
        nc, in_maps, core_ids=list(range(N_CORES)), trace=trace
    )
    LAST_RESULTS = res

    out = np.empty((E, OUT_F), np.float32)
    nck = E_LOC // OUT_CHUNK
    for c in range(N_CORES):
        o = res.results[c]["outd"]             # [nck, OUT_F, OUT_CHUNK]
        base = c * E_LOC
        for k in range(nck):
            out[base + k * OUT_CHUNK : base + (k + 1) * OUT_CHUNK] = o[k].T
    return out


# revision 28
# speedup vs baseline: 1.0106x; 1.0106x over previous
"""Trainium2 Bass kernel for nn_MessageFunction (gnn_message_passing).

Math (validated against the reference):
  The reference broadcasts h_w[:, :, None] -> (B*N, IN_F, N) and reshapes to
  [E, IN_F]; row-major order makes every row constant:
      h_w_rows[e, i] = h_w.reshape(-1)[e]   for all i.
  Hence the per-edge bmm collapses:
      m[e, o] = sum_i edge_output[e, o, i] * s[e]
              = s[e] * (x3[e] @ W4s[:, o] + b4s[o])
  with W4s = W4.reshape(HID3, OUT_F, IN_F).sum(-1), b4s = b4.reshape(OUT_F,
  IN_F).sum(-1), s = h_w.reshape(-1).  This is an exact reassociation (only
  f32 rounding differences) and removes the [E,128]@[128,4096] matmul + bmm.

Kernel: data-parallel over E = 32768 edges, 4096 per core across 8 cores,
MLP weights replicated, no cross-core communication.  Per core the MLP runs
features-on-partitions with edges streaming on the free dim:
    x1 = relu(W1.T @ eT)        K=32  -> [128, e]
    x2 = relu(W2.T @ x1)        K=128 -> [256, e] (two 128-part halves)
    x3 = relu(W3.T @ x2)        K=256 -> [128, e] (PSUM accumulation)
    y  = W4s.T @ x3             K=128 -> [64, e]  (col-packed 2 tiles/PSUM)
    out = (y + b4s) * s         one fused scalar_tensor_tensor on VectorE
Matmuls use float32r (full PE rate at N=512, near-fp32 precision).
"""

import os

import numpy as np

import concourse.bacc as bacc
import concourse.bass as bass
import concourse.mybir as mybir
import concourse.tile as tile
from concourse.bass_utils import run_bass_kernel_spmd

# Problem constants (hardcoded per the harness contract).
B, N = 8, 64
IN_F, OUT_F = 64, 64
EDGE_F = 32
HID1, HID2, HID3 = 128, 256, 128
E = B * N * N            # 32768
N_CORES = 8
E_LOC = E // N_CORES     # 4096
TILE = 512               # edges per tile (one PSUM bank per stage)
NT = E_LOC // TILE       # 8 tiles per core
OUT_CHUNK = 1024         # output DMA granularity (2 tiles)

F32 = mybir.dt.float32
# Matmul operand dtype: float32r streams at 1 cycle/row for N>=256 (same as
# bf16) with much better precision than bf16.
DT = mybir.dt.float32r
NP_DT = np.float32

# Module global: last BassKernelResults (test.py reads exec_time_ns from it).
LAST_RESULTS = None


def _build_bass(b2_halves_equal=True):
    nc = bacc.Bacc(
        "TRN2", target_bir_lowering=False, debug=False, num_devices=N_CORES
    )

    # Per-core inputs.  e_t is packed 4 edge-groups deep on partitions:
    # e_t[32*g + f, c] = e_vw[g*1024 + c, f] so one full-width DMA loads it.
    e_t = nc.dram_tensor("e_t", [128, E_LOC // 4], DT, kind="ExternalInput")
    s_b = nc.dram_tensor("s_b", [OUT_F, E_LOC], F32, kind="ExternalInput")
    # Replicated weights.  W1 is stacked 4x on partitions to serve the four
    # L1 row-tile positions.
    w1d = nc.dram_tensor("w1d", [128, HID1], DT, kind="ExternalInput")
    w2d = nc.dram_tensor("w2d", [HID1, HID2], DT, kind="ExternalInput")
    # W3 packed side by side: [:, 0:128] = W3[0:128, :], [:, 128:256] = W3[128:256, :]
    w3d = nc.dram_tensor("w3d", [128, 2 * HID3], DT, kind="ExternalInput")
    w4d = nc.dram_tensor("w4d", [HID3, OUT_F], DT, kind="ExternalInput")
    # Bias columns: b1, b2[:128], b2[128:], b3, [b4s; pad]
    bbd = nc.dram_tensor("bbd", [128, 5], F32, kind="ExternalInput")
    outd = nc.dram_tensor(
        "outd", [E_LOC // OUT_CHUNK, OUT_F, OUT_CHUNK], F32, kind="ExternalOutput"
    )

    # Relu pass engine schedule (per tile: L1, L2a, L2b, L3). 'A' = ScalarE,
    # 'V' = VectorE.  VectorE also runs the eight final bias+scale ops, so
    # ScalarE takes more of the 32 relu passes (20 A / 12 V).
    relu_sched = ["AVAV", "AVAA"] * (NT // 2)

    with tile.TileContext(nc) as tc:
        with (
            tc.tile_pool(name="wp", bufs=1) as wp,
            tc.tile_pool(name="io", bufs=4) as io,
            tc.tile_pool(name="acts", bufs=3) as acts,
            tc.tile_pool(name="ps", bufs=1, space="PSUM") as ps,
        ):
            e4 = wp.tile([128, E_LOC // 4], DT, tag="e4")
            w1 = wp.tile([128, HID1], DT, tag="w1")
            w2 = wp.tile([HID1, HID2], DT, tag="w2")
            w3 = wp.tile([128, 2 * HID3], DT, tag="w3")
            w4 = wp.tile([HID3, OUT_F], DT, tag="w4")
            bb = wp.tile([128, 5], F32, tag="bb")
            s_sb = wp.tile([OUT_F, E_LOC], F32, tag="s_sb")
            out_sb = wp.tile([OUT_F, E_LOC], F32, tag="out_sb")
            # Input loads spread over three issue rings so they parallelize:
            # Sync HWDGE carries the two tensors L1 needs first; GpSimd SWDGE
            # carries the remaining weights; Scalar HWDGE carries s.
            nc.sync.dma_start(e4[:], e_t[:])
            nc.sync.dma_start(w1[:], w1d[:])
            nc.gpsimd.dma_start(w2[:], w2d[:])
            nc.gpsimd.dma_start(w3[:], w3d[:])
            nc.gpsimd.dma_start(w4[:], w4d[:])
            nc.gpsimd.dma_start(bb[:], bbd[:])
            half = E_LOC // 2
            nc.scalar.dma_start(s_sb[:, 0:half], s_b[:, 0:half])
            nc.scalar.dma_start(s_sb[:, half:], s_b[:, half:])

            def relu_pass(dst, src, bias_col, eng):
                if eng == "A":
                    nc.scalar.activation(
                        dst, src, mybir.ActivationFunctionType.Relu, bias=bias_col
                    )
                else:
                    nc.vector.tensor_scalar(
                        out=dst,
                        in0=src,
                        scalar1=bias_col,
                        scalar2=0.0,
                        op0=mybir.AluOpType.add,
                        op1=mybir.AluOpType.max,
                    )

            # Software-pipelined emission, skewed so each pass result is
            # consumed one full iteration after it is produced — the PE
            # matmul stream never waits on a just-issued ScalarE/VectorE
            # pass.  Stage s of tile t runs in iteration t+s.
            x1_t = [None] * NT
            x2_t = [None] * NT
            x3_t = [None] * NT
            # P3 engine: 5 on ScalarE / 3 on VectorE (balance against the
            # fixed STT work on VectorE); P1 on VectorE, P2 on ScalarE.
            p3_eng = ["A", "V", "A", "A", "V", "A", "A", "V"]

            for i in range(NT + 4):
                # S1 + P1 for tile i.  L1 is a K=32 row-tiled matmul: edge
                # group g = i//2 lives on partitions [32g, 32g+32) of e4 and
                # w1 (stacked), with the matching tile_position row.
                if 0 <= i < NT:
                    g = i // 2
                    gp = slice(32 * g, 32 * g + 32)
                    gc = slice((i % 2) * TILE, (i % 2) * TILE + TILE)
                    x1p = ps.tile([128, TILE], F32, tag="x1p", bufs=2)
                    nc.tensor.matmul(
                        x1p[:], w1[gp, :], e4[gp, gc], tile_position=(32 * g, 0)
                    )
                    x1 = acts.tile([128, TILE], DT, tag="x1")
                    relu_pass(x1[:], x1p[:], bb[:, 0:1], "V")
                    x1_t[i] = x1

                # S2 + P2 for tile i-1 (merged 2-bank PSUM, single pass)
                j = i - 1
                if 0 <= j < NT:
                    x2p = ps.tile([128, 2 * TILE], F32, tag="x2p", bufs=1)
                    nc.tensor.matmul(x2p[:, 0:TILE], w2[:, 0:128], x1_t[j][:])
                    nc.tensor.matmul(x2p[:, TILE : 2 * TILE], w2[:, 128:256], x1_t[j][:])
                    x2 = acts.tile([128, 2 * TILE], DT, tag="x2")
                    # A per-partition bias is constant along the free dim, so
                    # one merged pass is only valid when both b2 halves agree
                    # (always true for the zero biases here); otherwise fall
                    # back to two passes.
                    if b2_halves_equal:
                        nc.scalar.activation(
                            x2[:], x2p[:],
                            mybir.ActivationFunctionType.Relu, bias=bb[:, 1:2],
                        )
                    else:
                        nc.scalar.activation(
                            x2[:, 0:TILE], x2p[:, 0:TILE],
                            mybir.ActivationFunctionType.Relu, bias=bb[:, 1:2],
                        )
                        nc.scalar.activation(
                            x2[:, TILE : 2 * TILE], x2p[:, TILE : 2 * TILE],
                            mybir.ActivationFunctionType.Relu, bias=bb[:, 2:3],
                        )
                    x2_t[j] = x2
                    x1_t[j] = None

                # S3 + P3 for tile i-2
                j = i - 2
                if 0 <= j < NT:
                    x3ps = ps.tile([128, TILE], F32, tag="x3ps", bufs=2)
                    nc.tensor.matmul(
                        x3ps[:], w3[:, 0:128], x2_t[j][:, 0:TILE],
                        start=True, stop=False,
                    )
                    nc.tensor.matmul(
                        x3ps[:], w3[:, 128:256], x2_t[j][:, TILE : 2 * TILE],
                        start=False, stop=True,
                    )
                    x3 = acts.tile([128, TILE], DT, tag="x3")
                    relu_pass(x3[:], x3ps[:], bb[:, 3:4], p3_eng[j])
                    x3_t[j] = x3
                    x2_t[j] = None

                # S4 + P4 for tile i-3
                j = i - 3
                if 0 <= j < NT:
                    cs = slice(j * TILE, (j + 1) * TILE)
                    y4p = ps.tile([OUT_F, TILE], F32, tag="y4p", bufs=2)
                    nc.tensor.matmul(y4p[:], w4[:], x3_t[j][:])
                    nc.vector.scalar_tensor_tensor(
                        out=out_sb[:, cs],
                        in0=y4p[:],
                        scalar=bb[0:OUT_F, 4:5],
                        in1=s_sb[:, cs],
                        op0=mybir.AluOpType.add,
                        op1=mybir.AluOpType.mult,
                    )
                    x3_t[j] = None
                    if (j + 1) * TILE % OUT_CHUNK == 0:
                        ck = ((j + 1) * TILE) // OUT_CHUNK - 1
                        nc.sync.dma_start(
                            outd[ck],
                            out_sb[:, ck * OUT_CHUNK : (ck + 1) * OUT_CHUNK],
                        )

    nc.compile()
    return nc


_CACHED_NC = None


def kernel(h_v, h_w, e_vw, W1, b1, W2, b2, W3, b3, W4, b4):
    global LAST_RESULTS, _CACHED_NC

    h_w = np.asarray(h_w, np.float32)
    e_vw = np.asarray(e_vw, np.float32)
    W1 = np.asarray(W1, np.float32)
    W2 = np.asarray(W2, np.float32)
    W3 = np.asarray(W3, np.float32)
    W4 = np.asarray(W4, np.float32)
    b1 = np.asarray(b1, np.float32)
    b2 = np.asarray(b2, np.float32)
    b3 = np.asarray(b3, np.float32)
    b4 = np.asarray(b4, np.float32)

    # Host-side weight transform (exact reassociation of the reference math).
    W4s = W4.reshape(HID3, OUT_F, IN_F).sum(axis=2)
    b4s = b4.reshape(OUT_F, IN_F).sum(axis=1)
    s = h_w.reshape(-1)

    w3p = np.concatenate([W3[0:128], W3[128:256]], axis=1)  # [128, 256]
    bb = np.zeros((128, 5), np.float32)
    bb[:, 0] = b1
    bb[:, 1] = b2[0:128]
    bb[:, 2] = b2[128:256]
    bb[:, 3] = b3
    bb[0:OUT_F, 4] = b4s

    weights_map = {
        "w1d": np.ascontiguousarray(np.tile(W1, (4, 1)), NP_DT),
        "w2d": np.ascontiguousarray(W2, NP_DT),
        "w3d": np.ascontiguousarray(w3p, NP_DT),
        "w4d": np.ascontiguousarray(W4s, NP_DT),
        "bbd": bb,
    }

    in_maps = []
    for c in range(N_CORES):
        sl = slice(c * E_LOC, (c + 1) * E_LOC)
        e_loc = e_vw[sl]                       # [4096, 32]
        s_loc = s[sl]                          # [4096]
        # [128, 1024]: partition 32g+f holds feature f of edge group g
        e_t = np.ascontiguousarray(
            e_loc.T.reshape(EDGE_F, 4, E_LOC // 4)
            .transpose(1, 0, 2)
            .reshape(128, E_LOC // 4),
            NP_DT,
        )
        s_bcast = np.ascontiguousarray(
            np.broadcast_to(s_loc[None, :], (OUT_F, E_LOC)), np.float32
        )
        in_maps.append({"e_t": e_t, "s_b": s_bcast, **weights_map})

    if _CACHED_NC is None:
        _CACHED_NC = _build_bass(
            b2_halves_equal=bool(np.array_equal(b2[0:128], b2[128:256]))
        )
    nc = _CACHED_NC

    trace = bool(int(os.environ.get("KERNEL_TRACE", "0")))
    res = run_bass_kernel_spmd(
        nc, in_maps, core_ids=list(range(N_CORES)), trace=trace
    )
    LAST_RESULTS = res

    out = np.empty((E, OUT_F), np.float32)
    nck = E_LOC // OUT_CHUNK
    for c in range(N_CORES):
        o = res.results[c]["outd"]             # [nck, OUT_F, OUT_CHUNK]
        base = c * E_LOC
        for k in range(nck):
            out[base + k * OUT_CHUNK : base + (k + 1) * OUT_CHUNK] = o[k].T
    return out


# revision 33
# speedup vs baseline: 1.0324x; 1.0216x over previous
"""Trainium2 Bass kernel for nn_MessageFunction (gnn_message_passing).

Math (validated against the reference):
  The reference broadcasts h_w[:, :, None] -> (B*N, IN_F, N) and reshapes to
  [E, IN_F]; row-major order makes every row constant:
      h_w_rows[e, i] = h_w.reshape(-1)[e]   for all i.
  Hence the per-edge bmm collapses:
      m[e, o] = sum_i edge_output[e, o, i] * s[e]
              = s[e] * (x3[e] @ W4s[:, o] + b4s[o])
  with W4s = W4.reshape(HID3, OUT_F, IN_F).sum(-1), b4s = b4.reshape(OUT_F,
  IN_F).sum(-1), s = h_w.reshape(-1).  This is an exact reassociation (only
  f32 rounding differences) and removes the [E,128]@[128,4096] matmul + bmm.

Kernel: data-parallel over E = 32768 edges, 4096 per core across 8 cores,
MLP weights replicated, no cross-core communication.  Per core the MLP runs
features-on-partitions with edges streaming on the free dim:
    x1 = relu(W1.T @ eT)        K=32  -> [128, e]
    x2 = relu(W2.T @ x1)        K=128 -> [256, e] (two 128-part halves)
    x3 = relu(W3.T @ x2)        K=256 -> [128, e] (PSUM accumulation)
    y  = W4s.T @ x3             K=128 -> [64, e]  (col-packed 2 tiles/PSUM)
    out = (y + b4s) * s         one fused scalar_tensor_tensor on VectorE
Matmuls use float32r (full PE rate at N=512, near-fp32 precision).
"""

import os

import numpy as np

import concourse.bacc as bacc
import concourse.bass as bass
import concourse.mybir as mybir
import concourse.tile as tile
from concourse.bass_utils import run_bass_kernel_spmd

# Problem constants (hardcoded per the harness contract).
B, N = 8, 64
IN_F, OUT_F = 64, 64
EDGE_F = 32
HID1, HID2, HID3 = 128, 256, 128
E = B * N * N            # 32768
N_CORES = 8
E_LOC = E // N_CORES     # 4096
TILE = 512               # edges per tile (one PSUM bank per stage)
NT = E_LOC // TILE       # 8 tiles per core
OUT_CHUNK = 1024         # output DMA granularity (2 tiles)

F32 = mybir.dt.float32
# Matmul operand dtype: float32r streams at 1 cycle/row for N>=256 (same as
# bf16) with much better precision than bf16.
DT = mybir.dt.float32r
NP_DT = np.float32

# Module global: last BassKernelResults (test.py reads exec_time_ns from it).
LAST_RESULTS = None


def _build_bass(b2_halves_equal=True):
    nc = bacc.Bacc(
        "TRN2", target_bir_lowering=False, debug=False, num_devices=N_CORES
    )

    # Per-core inputs.  e_t is packed 4 edge-groups deep on partitions:
    # e_t[32*g + f, c] = e_vw[g*1024 + c, f] so one full-width DMA loads it.
    e_t = nc.dram_tensor("e_t", [128, E_LOC // 4], DT, kind="ExternalInput")
    s_b = nc.dram_tensor("s_b", [OUT_F, E_LOC], F32, kind="ExternalInput")
    # Replicated weights.  W1 is stacked 4x on partitions to serve the four
    # L1 row-tile positions.
    w1d = nc.dram_tensor("w1d", [128, HID1], DT, kind="ExternalInput")
    w2d = nc.dram_tensor("w2d", [HID1, HID2], DT, kind="ExternalInput")
    # W3 packed side by side: [:, 0:128] = W3[0:128, :], [:, 128:256] = W3[128:256, :]
    w3d = nc.dram_tensor("w3d", [128, 2 * HID3], DT, kind="ExternalInput")
    w4d = nc.dram_tensor("w4d", [HID3, OUT_F], DT, kind="ExternalInput")
    # Bias columns: b1, b2[:128], b2[128:], b3, [b4s; pad]
    bbd = nc.dram_tensor("bbd", [128, 5], F32, kind="ExternalInput")
    outd = nc.dram_tensor(
        "outd", [E_LOC // OUT_CHUNK, OUT_F, OUT_CHUNK], F32, kind="ExternalOutput"
    )

    # Relu pass engine schedule (per tile: L1, L2a, L2b, L3). 'A' = ScalarE,
    # 'V' = VectorE.  VectorE also runs the eight final bias+scale ops, so
    # ScalarE takes more of the 32 relu passes (20 A / 12 V).
    relu_sched = ["AVAV", "AVAA"] * (NT // 2)

    with tile.TileContext(nc) as tc:
        with (
            tc.tile_pool(name="wp", bufs=1) as wp,
            tc.tile_pool(name="io", bufs=4) as io,
            tc.tile_pool(name="acts", bufs=3) as acts,
            tc.tile_pool(name="ps", bufs=1, space="PSUM") as ps,
        ):
            e4 = wp.tile([128, E_LOC // 4], DT, tag="e4")
            w1 = wp.tile([128, HID1], DT, tag="w1")
            w2 = wp.tile([HID1, HID2], DT, tag="w2")
            w3 = wp.tile([128, 2 * HID3], DT, tag="w3")
            w4 = wp.tile([HID3, OUT_F], DT, tag="w4")
            bb = wp.tile([128, 5], F32, tag="bb")
            s_sb = wp.tile([OUT_F, E_LOC], F32, tag="s_sb")
            out_sb = wp.tile([OUT_F, E_LOC], F32, tag="out_sb")
            # Input loads on the two HWDGE rings (Sync + Scalar), which issue
            # in ~0.6us and stream FIFO; the GpSimd SWDGE path costs ~2us
            # fixed per transfer, far too slow for the startup path.
            nc.sync.dma_start(w1[:], w1d[:])
            nc.sync.dma_start(e4[:], e_t[:])
            nc.sync.dma_start(w2[:], w2d[:])
            nc.sync.dma_start(w3[:], w3d[:])
            nc.sync.dma_start(w4[:], w4d[:])
            nc.sync.dma_start(bb[:], bbd[:])
            half = E_LOC // 2
            nc.scalar.dma_start(s_sb[:, 0:half], s_b[:, 0:half])
            nc.scalar.dma_start(s_sb[:, half:], s_b[:, half:])

            # PE warm-up: ~10 dependency-free matmuls on scratch data run
            # back-to-back during the input-load window, so the HAM clock
            # gate reaches 2.4 GHz before the first real matmul.  Garbage
            # values are fine — the scratch PSUM is never read.
            scratch = wp.tile([128, TILE], DT, tag="scratch")
            nc.gpsimd.memset(scratch[:].bitcast(F32), 1.0)
            warm_ps = ps.tile([128, TILE], F32, tag="y4p", bufs=2)
            for _ in range(10):
                nc.tensor.matmul(warm_ps[:], scratch[:, 0:128], scratch[:])

            def relu_pass(dst, src, bias_col, eng):
                if eng == "A":
                    nc.scalar.activation(
                        dst, src, mybir.ActivationFunctionType.Relu, bias=bias_col
                    )
                else:
                    nc.vector.tensor_scalar(
                        out=dst,
                        in0=src,
                        scalar1=bias_col,
                        scalar2=0.0,
                        op0=mybir.AluOpType.add,
                        op1=mybir.AluOpType.max,
                    )

            # Software-pipelined emission, skewed so each pass result is
            # consumed one full iteration after it is produced — the PE
            # matmul stream never waits on a just-issued ScalarE/VectorE
            # pass.  Stage s of tile t runs in iteration t+s.
            x1_t = [None] * NT
            x2_t = [None] * NT
            x3_t = [None] * NT
            # P3 engine: 6 on ScalarE / 2 on VectorE (balance against the
            # fixed STT work on VectorE); P1 on VectorE, P2 on ScalarE.
            p3_eng = ["A", "A", "V", "A", "A", "A", "V", "A"]

            for i in range(NT + 4):
                # S1 + P1 for tile i.  L1 is a K=32 row-tiled matmul: edge
                # group g = i//2 lives on partitions [32g, 32g+32) of e4 and
                # w1 (stacked), with the matching tile_position row.
                if 0 <= i < NT:
                    g = i // 2
                    gp = slice(32 * g, 32 * g + 32)
                    gc = slice((i % 2) * TILE, (i % 2) * TILE + TILE)
                    x1p = ps.tile([128, TILE], F32, tag="x1p", bufs=2)
                    nc.tensor.matmul(
                        x1p[:], w1[gp, :], e4[gp, gc], tile_position=(32 * g, 0)
                    )
                    x1 = acts.tile([128, TILE], DT, tag="x1")
                    relu_pass(x1[:], x1p[:], bb[:, 0:1], "V")
                    x1_t[i] = x1

                # S2 + P2 for tile i-1 (merged 2-bank PSUM, single pass)
                j = i - 1
                if 0 <= j < NT:
                    x2p = ps.tile([128, 2 * TILE], F32, tag="x2p", bufs=1)
                    nc.tensor.matmul(x2p[:, 0:TILE], w2[:, 0:128], x1_t[j][:])
                    nc.tensor.matmul(x2p[:, TILE : 2 * TILE], w2[:, 128:256], x1_t[j][:])
                    x2 = acts.tile([128, 2 * TILE], DT, tag="x2")
                    # A per-partition bias is constant along the free dim, so
                    # one merged pass is only valid when both b2 halves agree
                    # (always true for the zero biases here); otherwise fall
                    # back to two passes.
                    if b2_halves_equal:
                        nc.scalar.activation(
                            x2[:], x2p[:],
                            mybir.ActivationFunctionType.Relu, bias=bb[:, 1:2],
                        )
                    else:
                        nc.scalar.activation(
                            x2[:, 0:TILE], x2p[:, 0:TILE],
                            mybir.ActivationFunctionType.Relu, bias=bb[:, 1:2],
                        )
                        nc.scalar.activation(
                            x2[:, TILE : 2 * TILE], x2p[:, TILE : 2 * TILE],
                            mybir.ActivationFunctionType.Relu, bias=bb[:, 2:3],
                        )
                    x2_t[j] = x2
                    x1_t[j] = None

                # S3 + P3 for tile i-2
                j = i - 2
                if 0 <= j < NT:
                    x3ps = ps.tile([128, TILE], F32, tag="x3ps", bufs=2)
                    nc.tensor.matmul(
                        x3ps[:], w3[:, 0:128], x2_t[j][:, 0:TILE],
                        start=True, stop=False,
                    )
                    nc.tensor.matmul(
                        x3ps[:], w3[:, 128:256], x2_t[j][:, TILE : 2 * TILE],
                        start=False, stop=True,
                    )
                    x3 = acts.tile([128, TILE], DT, tag="x3")
                    relu_pass(x3[:], x3ps[:], bb[:, 3:4], p3_eng[j])
                    x3_t[j] = x3
                    x2_t[j] = None

                # S4 + P4 for tile i-3
                j = i - 3
                if 0 <= j < NT:
                    cs = slice(j * TILE, (j + 1) * TILE)
                    y4p = ps.tile([OUT_F, TILE], F32, tag="y4p", bufs=2)
                    nc.tensor.matmul(y4p[:], w4[:], x3_t[j][:])
                    nc.vector.scalar_tensor_tensor(
                        out=out_sb[:, cs],
                        in0=y4p[:],
                        scalar=bb[0:OUT_F, 4:5],
                        in1=s_sb[:, cs],
                        op0=mybir.AluOpType.add,
                        op1=mybir.AluOpType.mult,
                    )
                    x3_t[j] = None
                    if (j + 1) * TILE % OUT_CHUNK == 0:
                        ck = ((j + 1) * TILE) // OUT_CHUNK - 1
                        nc.sync.dma_start(
                            outd[ck],
                            out_sb[:, ck * OUT_CHUNK : (ck + 1) * OUT_CHUNK],
                        )

    nc.compile()
    return nc


_CACHED_NC = None


def kernel(h_v, h_w, e_vw, W1, b1, W2, b2, W3, b3, W4, b4):
    global LAST_RESULTS, _CACHED_NC

    h_w = np.asarray(h_w, np.float32)
    e_vw = np.asarray(e_vw, np.float32)
    W1 = np.asarray(W1, np.float32)
    W2 = np.asarray(W2, np.float32)
    W3 = np.asarray(W3, np.float32)
    W4 = np.asarray(W4, np.float32)
    b1 = np.asarray(b1, np.float32)
    b2 = np.asarray(b2, np.float32)
    b3 = np.asarray(b3, np.float32)
    b4 = np.asarray(b4, np.float32)

    # Host-side weight transform (exact reassociation of the reference math).
    W4s = W4.reshape(HID3, OUT_F, IN_F).sum(axis=2)
    b4s = b4.reshape(OUT_F, IN_F).sum(axis=1)
    s = h_w.reshape(-1)

    w3p = np.concatenate([W3[0:128], W3[128:256]], axis=1)  # [128, 256]
    bb = np.zeros((128, 5), np.float32)
    bb[:, 0] = b1
    bb[:, 1] = b2[0:128]
    bb[:, 2] = b2[128:256]
    bb[:, 3] = b3
    bb[0:OUT_F, 4] = b4s

    weights_map = {
        "w1d": np.ascontiguousarray(np.tile(W1, (4, 1)), NP_DT),
        "w2d": np.ascontiguousarray(W2, NP_DT),
        "w3d": np.ascontiguousarray(w3p, NP_DT),
        "w4d": np.ascontiguousarray(W4s, NP_DT),
        "bbd": bb,
    }

    in_maps = []
    for c in range(N_CORES):
        sl = slice(c * E_LOC, (c + 1) * E_LOC)
        e_loc = e_vw[sl]                       # [4096, 32]
        s_loc = s[sl]                          # [4096]
        # [128, 1024]: partition 32g+f holds feature f of edge group g
        e_t = np.ascontiguousarray(
            e_loc.T.reshape(EDGE_F, 4, E_LOC // 4)
            .transpose(1, 0, 2)
            .reshape(128, E_LOC // 4),
            NP_DT,
        )
        s_bcast = np.ascontiguousarray(
            np.broadcast_to(s_loc[None, :], (OUT_F, E_LOC)), np.float32
        )
        in_maps.append({"e_t": e_t, "s_b": s_bcast, **weights_map})

    if _CACHED_NC is None:
        _CACHED_NC = _build_bass(
            b2_halves_equal=bool(np.array_equal(b2[0:128], b2[128:256]))
        )
    nc = _CACHED_NC

    trace = bool(int(os.environ.get("KERNEL_TRACE", "0")))
    res = run_bass_kernel_spmd(
        nc, in_maps, core_ids=list(range(N_CORES)), trace=trace
    )
    LAST_RESULTS = res

    out = np.empty((E, OUT_F), np.float32)
    nck = E_LOC // OUT_CHUNK
    for c in range(N_CORES):
        o = res.results[c]["outd"]             # [nck, OUT_F, OUT_CHUNK]
        base = c * E_LOC
        for k in range(nck):
            out[base + k * OUT_CHUNK : base + (k + 1) * OUT_CHUNK] = o[k].T
    return out


# revision 34
# speedup vs baseline: 1.1989x; 1.1612x over previous
"""Trainium2 Bass kernel for nn_MessageFunction (gnn_message_passing).

Math (validated against the reference):
  The reference broadcasts h_w[:, :, None] -> (B*N, IN_F, N) and reshapes to
  [E, IN_F]; row-major order makes every row constant:
      h_w_rows[e, i] = h_w.reshape(-1)[e]   for all i.
  Hence the per-edge bmm collapses:
      m[e, o] = sum_i edge_output[e, o, i] * s[e]
              = s[e] * (x3[e] @ W4s[:, o] + b4s[o])
  with W4s = W4.reshape(HID3, OUT_F, IN_F).sum(-1), b4s = b4.reshape(OUT_F,
  IN_F).sum(-1), s = h_w.reshape(-1).  This is an exact reassociation (only
  f32 rounding differences) and removes the [E,128]@[128,4096] matmul + bmm.

Kernel: data-parallel over E = 32768 edges, 4096 per core across 8 cores,
MLP weights replicated, no cross-core communication.  Per core the MLP runs
features-on-partitions with edges streaming on the free dim:
    x1 = relu(W1.T @ eT)        K=32  -> [128, e]
    x2 = relu(W2.T @ x1)        K=128 -> [256, e] (two 128-part halves)
    x3 = relu(W3.T @ x2)        K=256 -> [128, e] (PSUM accumulation)
    y  = W4s.T @ x3             K=128 -> [64, e]  (col-packed 2 tiles/PSUM)
    out = (y + b4s) * s         one fused scalar_tensor_tensor on VectorE
Matmuls use float32r (full PE rate at N=512, near-fp32 precision).
"""

import os

import numpy as np

import concourse.bacc as bacc
import concourse.bass as bass
import concourse.mybir as mybir
import concourse.tile as tile
from concourse.bass_utils import run_bass_kernel_spmd

# Problem constants (hardcoded per the harness contract).
B, N = 8, 64
IN_F, OUT_F = 64, 64
EDGE_F = 32
HID1, HID2, HID3 = 128, 256, 128
E = B * N * N            # 32768
N_CORES = 8
E_LOC = E // N_CORES     # 4096
TILE = 512               # edges per tile (one PSUM bank per stage)
NT = E_LOC // TILE       # 8 tiles per core
OUT_CHUNK = 1024         # output DMA granularity (2 tiles)

F32 = mybir.dt.float32
# Matmul operand dtype: float32r streams at 1 cycle/row for N>=256 (same as
# bf16) with much better precision than bf16.
DT = mybir.dt.float32r
NP_DT = np.float32

# Module global: last BassKernelResults (test.py reads exec_time_ns from it).
LAST_RESULTS = None


def _build_bass(b2_halves_equal=True):
    nc = bacc.Bacc(
        "TRN2", target_bir_lowering=False, debug=False, num_devices=N_CORES
    )

    # Per-core inputs.  e_t is packed 4 edge-groups deep on partitions:
    # e_t[32*g + f, c] = e_vw[g*1024 + c, f] so one full-width DMA loads it.
    e_t = nc.dram_tensor("e_t", [128, E_LOC // 4], DT, kind="ExternalInput")
    s_b = nc.dram_tensor("s_b", [OUT_F, E_LOC], F32, kind="ExternalInput")
    # Replicated weights.  W1 is stacked 4x on partitions to serve the four
    # L1 row-tile positions.
    w1d = nc.dram_tensor("w1d", [128, HID1], DT, kind="ExternalInput")
    w2d = nc.dram_tensor("w2d", [HID1, HID2], DT, kind="ExternalInput")
    # W3 packed side by side: [:, 0:128] = W3[0:128, :], [:, 128:256] = W3[128:256, :]
    w3d = nc.dram_tensor("w3d", [128, 2 * HID3], DT, kind="ExternalInput")
    w4d = nc.dram_tensor("w4d", [HID3, OUT_F], DT, kind="ExternalInput")
    # Bias columns: b1, b2[:128], b2[128:], b3, [b4s; pad]
    bbd = nc.dram_tensor("bbd", [128, 5], F32, kind="ExternalInput")
    outd = nc.dram_tensor(
        "outd", [E_LOC // OUT_CHUNK, OUT_F, OUT_CHUNK], F32, kind="ExternalOutput"
    )

    # Relu pass engine schedule (per tile: L1, L2a, L2b, L3). 'A' = ScalarE,
    # 'V' = VectorE.  VectorE also runs the eight final bias+scale ops, so
    # ScalarE takes more of the 32 relu passes (20 A / 12 V).
    relu_sched = ["AVAV", "AVAA"] * (NT // 2)

    with tile.TileContext(nc) as tc:
        with (
            tc.tile_pool(name="wp", bufs=1) as wp,
            tc.tile_pool(name="io", bufs=4) as io,
            tc.tile_pool(name="acts", bufs=3) as acts,
            tc.tile_pool(name="ps", bufs=1, space="PSUM") as ps,
        ):
            e4 = wp.tile([128, E_LOC // 4], DT, tag="e4")
            w1 = wp.tile([128, HID1], DT, tag="w1")
            w2 = wp.tile([HID1, HID2], DT, tag="w2")
            w3 = wp.tile([128, 2 * HID3], DT, tag="w3")
            w4 = wp.tile([HID3, OUT_F], DT, tag="w4")
            bb = wp.tile([128, 5], F32, tag="bb")
            s_sb = wp.tile([OUT_F, E_LOC], F32, tag="s_sb")
            out_sb = wp.tile([OUT_F, E_LOC], F32, tag="out_sb")
            # Input loads on the two HWDGE rings (Sync + Scalar), which issue
            # in ~0.6us and stream FIFO; the GpSimd SWDGE path costs ~2us
            # fixed per transfer, far too slow for the startup path.
            # Sync ring carries ONLY what the first L1 matmul needs (ring is
            # FIFO end-to-end, so anything else here delays the whole kernel).
            nc.sync.dma_start(w1[:], w1d[:])
            nc.sync.dma_start(e4[:], e_t[:])
            nc.scalar.dma_start(bb[:], bbd[:])
            nc.scalar.dma_start(w2[:], w2d[:])
            nc.scalar.dma_start(w3[:], w3d[:])
            nc.scalar.dma_start(w4[:], w4d[:])
            half = E_LOC // 2
            nc.scalar.dma_start(s_sb[:, 0:half], s_b[:, 0:half])
            nc.scalar.dma_start(s_sb[:, half:], s_b[:, half:])

            # PE warm-up: ~10 dependency-free matmuls on scratch data run
            # back-to-back during the input-load window, so the HAM clock
            # gate reaches 2.4 GHz before the first real matmul.  Garbage
            # values are fine — the scratch PSUM is never read.
            scratch = wp.tile([128, TILE], DT, tag="scratch")
            nc.gpsimd.memset(scratch[:].bitcast(F32), 1.0)
            warm_ps = ps.tile([128, TILE], F32, tag="y4p", bufs=2)
            for _ in range(10):
                nc.tensor.matmul(warm_ps[:], scratch[:, 0:128], scratch[:])

            def relu_pass(dst, src, bias_col, eng):
                if eng == "A":
                    nc.scalar.activation(
                        dst, src, mybir.ActivationFunctionType.Relu, bias=bias_col
                    )
                else:
                    nc.vector.tensor_scalar(
                        out=dst,
                        in0=src,
                        scalar1=bias_col,
                        scalar2=0.0,
                        op0=mybir.AluOpType.add,
                        op1=mybir.AluOpType.max,
                    )

            # Software-pipelined emission, skewed so each pass result is
            # consumed one full iteration after it is produced — the PE
            # matmul stream never waits on a just-issued ScalarE/VectorE
            # pass.  Stage s of tile t runs in iteration t+s.
            x1_t = [None] * NT
            x2_t = [None] * NT
            x3_t = [None] * NT
            # P3 engine: 6 on ScalarE / 2 on VectorE (balance against the
            # fixed STT work on VectorE); P1 on VectorE, P2 on ScalarE.
            p3_eng = ["A", "A", "V", "A", "A", "A", "V", "A"]

            for i in range(NT + 4):
                # S1 + P1 for tile i.  L1 is a K=32 row-tiled matmul: edge
                # group g = i//2 lives on partitions [32g, 32g+32) of e4 and
                # w1 (stacked), with the matching tile_position row.
                if 0 <= i < NT:
                    g = i // 2
                    gp = slice(32 * g, 32 * g + 32)
                    gc = slice((i % 2) * TILE, (i % 2) * TILE + TILE)
                    x1p = ps.tile([128, TILE], F32, tag="x1p", bufs=2)
                    nc.tensor.matmul(
                        x1p[:], w1[gp, :], e4[gp, gc], tile_position=(32 * g, 0)
                    )
                    x1 = acts.tile([128, TILE], DT, tag="x1")
                    relu_pass(x1[:], x1p[:], bb[:, 0:1], "V")
                    x1_t[i] = x1

                # S2 + P2 for tile i-1 (merged 2-bank PSUM, single pass)
                j = i - 1
                if 0 <= j < NT:
                    x2p = ps.tile([128, 2 * TILE], F32, tag="x2p", bufs=1)
                    nc.tensor.matmul(x2p[:, 0:TILE], w2[:, 0:128], x1_t[j][:])
                    nc.tensor.matmul(x2p[:, TILE : 2 * TILE], w2[:, 128:256], x1_t[j][:])
                    x2 = acts.tile([128, 2 * TILE], DT, tag="x2")
                    # A per-partition bias is constant along the free dim, so
                    # one merged pass is only valid when both b2 halves agree
                    # (always true for the zero biases here); otherwise fall
                    # back to two passes.
                    if b2_halves_equal:
                        nc.scalar.activation(
                            x2[:], x2p[:],
                            mybir.ActivationFunctionType.Relu, bias=bb[:, 1:2],
                        )
                    else:
                        nc.scalar.activation(
                            x2[:, 0:TILE], x2p[:, 0:TILE],
                            mybir.ActivationFunctionType.Relu, bias=bb[:, 1:2],
                        )
                        nc.scalar.activation(
                            x2[:, TILE : 2 * TILE], x2p[:, TILE : 2 * TILE],
                            mybir.ActivationFunctionType.Relu, bias=bb[:, 2:3],
                        )
                    x2_t[j] = x2
                    x1_t[j] = None

                # S3 + P3 for tile i-2
                j = i - 2
                if 0 <= j < NT:
                    x3ps = ps.tile([128, TILE], F32, tag="x3ps", bufs=2)
                    nc.tensor.matmul(
                        x3ps[:], w3[:, 0:128], x2_t[j][:, 0:TILE],
                        start=True, stop=False,
                    )
                    nc.tensor.matmul(
                        x3ps[:], w3[:, 128:256], x2_t[j][:, TILE : 2 * TILE],
                        start=False, stop=True,
                    )
                    x3 = acts.tile([128, TILE], DT, tag="x3")
                    relu_pass(x3[:], x3ps[:], bb[:, 3:4], p3_eng[j])
                    x3_t[j] = x3
                    x2_t[j] = None

                # S4 + P4 for tile i-3
                j = i - 3
                if 0 <= j < NT:
                    cs = slice(j * TILE, (j + 1) * TILE)
                    y4p = ps.tile([OUT_F, TILE], F32, tag="y4p", bufs=2)
                    nc.tensor.matmul(y4p[:], w4[:], x3_t[j][:])
                    nc.vector.scalar_tensor_tensor(
                        out=out_sb[:, cs],
                        in0=y4p[:],
                        scalar=bb[0:OUT_F, 4:5],
                        in1=s_sb[:, cs],
                        op0=mybir.AluOpType.add,
                        op1=mybir.AluOpType.mult,
                    )
                    x3_t[j] = None
                    if (j + 1) * TILE % OUT_CHUNK == 0:
                        ck = ((j + 1) * TILE) // OUT_CHUNK - 1
                        nc.sync.dma_start(
                            outd[ck],
                            out_sb[:, ck * OUT_CHUNK : (ck + 1) * OUT_CHUNK],
                        )

    nc.compile()
    return nc


_CACHED_NC = None


def kernel(h_v, h_w, e_vw, W1, b1, W2, b2, W3, b3, W4, b4):
    global LAST_RESULTS, _CACHED_NC

    h_w = np.asarray(h_w, np.float32)
    e_vw = np.asarray(e_vw, np.float32)
    W1 = np.asarray(W1, np.float32)
    W2 = np.asarray(W2, np.float32)
    W3 = np.asarray(W3, np.float32)
    W4 = np.asarray(W4, np.float32)
    b1 = np.asarray(b1, np.float32)
    b2 = np.asarray(b2, np.float32)
    b3 = np.asarray(b3, np.float32)
    b4 = np.asarray(b4, np.float32)

    # Host-side weight transform (exact reassociation of the reference math).
    W4s = W4.reshape(HID3, OUT_F, IN_F).sum(axis=2)
    b4s = b4.reshape(OUT_F, IN_F).sum(axis=1)
    s = h_w.reshape(-1)

    w3p = np.concatenate([W3[0:128], W3[128:256]], axis=1)  # [128, 256]
    bb = np.zeros((128, 5), np.float32)
    bb[:, 0] = b1
    bb[:, 1] = b2[0:128]
    bb[:, 2] = b2[128:256]
    bb[:, 3] = b3
    bb[0:OUT_F, 4] = b4s

    weights_map = {
        "w1d": np.ascontiguousarray(np.tile(W1, (4, 1)), NP_DT),
        "w2d": np.ascontiguousarray(W2, NP_DT),
        "w3d": np.ascontiguousarray(w3p, NP_DT),
        "w4d": np.ascontiguousarray(W4s, NP_DT),
        "bbd": bb,
    }

    in_maps = []
    for c in range(N_CORES):
        sl = slice(c * E_LOC, (c + 1) * E_LOC)
        e_loc = e_vw[sl]                       # [4096, 32]
        s_loc = s[sl]                          # [4096]
        # [128, 1024]: partition 32g+f holds feature f of edge group g
        e_t = np.ascontiguousarray(
            e_loc.T.reshape(EDGE_F, 4, E_LOC // 4)
            .transpose(1, 0, 2)
            .reshape(128, E_LOC // 4),
            NP_DT,
        )
        s_bcast = np.ascontiguousarray(
            np.broadcast_to(s_loc[None, :], (OUT_F, E_LOC)), np.float32
        )
        in_maps.append({"e_t": e_t, "s_b": s_bcast, **weights_map})

    if _CACHED_NC is None:
        _CACHED_NC = _build_bass(
            b2_halves_equal=bool(np.array_equal(b2[0:128], b2[128:256]))
        )
    nc = _CACHED_NC

    trace = bool(int(os.environ.get("KERNEL_TRACE", "0")))
    res = run_bass_kernel_spmd(
        nc, in_maps, core_ids=list(range(N_CORES)), trace=trace
    )
    LAST_RESULTS = res

    out = np.empty((E, OUT_F), np.float32)
    nck = E_LOC // OUT_CHUNK
    for c in range(N_CORES):
        o = res.results[c]["outd"]             # [nck, OUT_F, OUT_CHUNK]
        base = c * E_LOC
        for k in range(nck):
            out[base + k * OUT_CHUNK : base + (k + 1) * OUT_CHUNK] = o[k].T
    return out
